# revision 47
# baseline (speedup 1.0000x reference)
"""GAU (gated attention unit) forward for Trainium2, 8 NeuronCores data-parallel.

Contract: kernel(**inputs) takes the FULL unsharded inputs (as produced by the
problem's setup_inputs) and returns the FULL [32, 512, 512] float32 output.

Strategy: pure data parallelism over batch (32 = 8 cores x 4 elements).  All
large matmuls run in fp8 (e4m3) with DoubleRow perf mode at 512-wide moving
operands.  Power-of-two scale factors keep every fp8/bf16 intermediate in
range; the compensating multiply is fused into the final PSUM eviction.  The
residual shortcut stays fp32 end to end.

Engine split per element (engine-balanced; GpSimd has no PSUM port; walrus
only accepts the classic op set: matmul/activation/TT/TS/copy):
  ACT   RMS stats (Square+accum), silu evictions (1024-wide pairs), score
        eviction A (scaled copy), tail out-evictions of the last elements
  DVE   rope z/relu (4x TS), Newton rsqrt, score eviction B, gating (TT:
        AV psum x uT -> fp8), out descale (TS), hT fp8 eviction
  Pool  bf16 cast of x, rsqrt diag build, q/k affine, rope cos/sin muls,
        squared-relu (TT z*relu(z) -> fp8), shortcut add
  PE    matmuls only (512-wide fp8 DoubleRow); the h transpose is a plain
        bf16 matmul against diag(rsqrt) so the normalization scale rides
        the transpose for free; dummy matmuls during the DMA fill release
        the HAM clock throttle before real work.

Pipeline: PSUM is split into two independent 2x2-bank rings (proj vs
attention-output) so B(i) never queues behind C(i-2); A-phases are hoisted
one element ahead; constant DMAs ride the SP queue ordered by first use;
the tail interleaves the last elements' C phases into B(3) with v-before-u
ordering and quarter-granular output stores.
"""

import os
import sys

for _p in ("/opt/trn_rl_repo",):
    if _p not in sys.path:
        sys.path.insert(0, _p)

import numpy as np

import concourse.bass as bass
import concourse.mybir as mybir
import concourse.tile as tile
from concourse.bass_utils import run_bass_kernel_spmd
from concourse.masks import make_identity

F32 = mybir.dt.float32
BF16 = mybir.dt.bfloat16
FP8 = mybir.dt.float8e4
U32 = mybir.dt.uint32
U16 = mybir.dt.uint16
U8 = mybir.dt.uint8

P = 128          # partitions
N = 512          # seq len
D = 512          # model dim
E = 1024         # expand dim
S = 128          # shared q/k dim
PROJ = 2 * E + S  # 2176
PER = 4          # batch elements per core
CORES = 8
EPS = 1e-6
ACT = mybir.ActivationFunctionType
ALU = mybir.AluOpType
PM = mybir.MatmulPerfMode
RSQRT_MAGIC = 0x5F3759DF

NCH = N // P     # 4 seq chunks
DCH = D // P     # 4 model-dim chunks
ECH = E // P     # 8 expand chunks
UBCH = (E + S) // P  # 9 transposed u+base chunks

# power-of-two scale ladder (see _prep_inputs)
W1_SCALE = 2.0 ** 6     # on W1 (and b1) so fp8 entries are ~N(0,1.3)
QK_LAM = 2.0 ** 7       # on gamma/beta so q/k fp8 entries are O(1)
SQ_SCALE = 2.0 ** -5    # folded into the score eviction: qkAB = qk_true*2^18
W2_SCALE = 2.0 ** 6     # on W2
OUT_DESCALE = 2.0 ** -42  # 2^36 (kernel) * 2^6 (W2) compensated at the end

SILU_FUNC = ACT.Silu


def _build_program(b1_nonzero: bool) -> bass.Bass:
    nc = bass.Bass(trn_type="TRN2")

    x_d = nc.dram_tensor("x", [PER, N, D], F32, kind="ExternalInput")
    # W1 u+base columns, DoubleRow stationary: [p, ks, t, mc, 128]
    w1u_d = nc.dram_tensor("w1u", [P, 2, 2, UBCH, P], U8, kind="ExternalInput")
    # W1 v columns, DoubleRow moving: [p, ks, t, eh, 512]
    w1v_d = nc.dram_tensor("w1v", [P, 2, 2, 2, 512], U8, kind="ExternalInput")
    # W2, DoubleRow moving: [p, ks, t, 512]
    w2_d = nc.dram_tensor("w2", [P, 4, 2, 512], U8, kind="ExternalInput")
    b1t_d = nc.dram_tensor("b1t", [P, UBCH], F32, kind="ExternalInput")
    qkg_d = nc.dram_tensor("qkg", [P, 4], F32, kind="ExternalInput")
    # rope tables: cs1 = [cos0, cos1, sin0, sin1], cs2 = [sin0, sin1, cos0, cos1]
    cs1_d = nc.dram_tensor("cs1", [P, 4, N], U16, kind="ExternalInput")
    cs2_d = nc.dram_tensor("cs2", [P, 4, N], U16, kind="ExternalInput")
    b1v_d = nc.dram_tensor("b1v", [1, E], U8, kind="ExternalInput") if b1_nonzero else None
    out_d = nc.dram_tensor("out", [PER, N, D], F32, kind="ExternalOutput")

    with tile.TileContext(nc) as tc:
        with (
            tc.tile_pool(name="consts", bufs=1) as consts,
            tc.tile_pool(name="xp", bufs=4) as xp,
            tc.tile_pool(name="h8p", bufs=2) as h8p,
            tc.tile_pool(name="dgp", bufs=2) as dgp,
            tc.tile_pool(name="htp", bufs=2) as htp,
            tc.tile_pool(name="utp", bufs=3) as utp,
            tc.tile_pool(name="basep", bufs=2) as basep,
            tc.tile_pool(name="qkabp", bufs=2) as qkabp,
            tc.tile_pool(name="ropep", bufs=2) as ropep,
            tc.tile_pool(name="ktp", bufs=2) as ktp,
            tc.tile_pool(name="vp", bufs=3) as vp,
            tc.tile_pool(name="gtp", bufs=2) as gtp,
            tc.tile_pool(name="op", bufs=2) as op,
            tc.tile_pool(name="nstat", bufs=2) as nstat,
            # PSUM: decoupled rings so the proj path (B) never waits on
            # the attention-output path (C): 2 pools x 2 bufs x 2 banks
            tc.tile_pool(name="bp", bufs=2, space="PSUM") as bp,
            tc.tile_pool(name="cp", bufs=2, space="PSUM") as cp,
        ):
            # ---- constants (tiles only; DMAs are issued inside the
            # schedule on the SP queue, ordered by first use, so they never
            # block the compute engines' queues) ----
            ident = consts.tile([P, P], BF16)
            make_identity(nc, ident[:])
            # warm the PE clock gate during the DMA fill: ~3us of dummy
            # matmul activity releases the HAM throttle before real work
            warmps = cp.tile([P, 2, 512], F32, name="warmps", tag="c")
            for _ in range(24):
                nc.tensor.matmul(
                    warmps[:, 0, 0:P], lhsT=ident[:], rhs=ident[:],
                    start=True, stop=True,
                )
            w1u_sb = consts.tile([P, 2, 2, UBCH, P], FP8)
            w1v_sb = consts.tile([P, 2, 2, 2, 512], FP8)
            w2sb = consts.tile([P, 4, 2, 512], FP8)
            b1t_sb = consts.tile([P, UBCH], F32)
            qkg_sb = consts.tile([P, 4], F32)
            cs1_sb = consts.tile([P, 4, N], BF16)
            cs2_sb = consts.tile([P, 4, N], BF16)
            magic_sb = consts.tile([P, NCH], U32)
            nc.vector.memset(magic_sb[:], RSQRT_MAGIC)

            if b1_nonzero:
                ones_sb = consts.tile([1, P], FP8)
                nc.vector.memset(ones_sb[:], 1.0)
                b1v_sb = consts.tile([1, E], FP8)

            def load_consts(group):
                if group == 1:
                    nc.sync.dma_start(qkg_sb[:], qkg_d[:])
                    nc.sync.dma_start(w1u_sb[:].bitcast(U8), w1u_d[:])
                elif group == 2:
                    nc.sync.dma_start(w1v_sb[:].bitcast(U8), w1v_d[:])
                    nc.sync.dma_start(cs1_sb[:].bitcast(U16), cs1_d[:])
                    nc.sync.dma_start(cs2_sb[:].bitcast(U16), cs2_d[:])
                else:
                    nc.sync.dma_start(w2sb[:].bitcast(U8), w2_d[:])
                    nc.sync.dma_start(b1t_sb[:], b1t_d[:])
                    if b1_nonzero:
                        nc.sync.dma_start(b1v_sb[:].bitcast(U8), b1v_d[:])

            st = {}

            def _rsqrt(ms, a_t, y_t, nt, sl):
                # rs = 1/sqrt(ms/D + eps): fast-inv-sqrt + 1 Newton (DVE;
                # the shift/int variants are not walrus-legal on Pool)
                nc.vector.tensor_scalar(a_t[sl], ms[sl], 1.0 / D, EPS, ALU.mult, ALU.add)
                nc.vector.tensor_scalar(
                    y_t[sl].bitcast(U32), a_t[sl].bitcast(U32), 1, None,
                    ALU.logical_shift_right,
                )
                nc.vector.tensor_sub(
                    y_t[sl].bitcast(U32), magic_sb[sl], y_t[sl].bitcast(U32)
                )
                nc.vector.tensor_mul(nt[sl], a_t[sl], y_t[sl])
                nc.vector.tensor_mul(nt[sl], nt[sl], y_t[sl])
                nc.vector.tensor_scalar(nt[sl], nt[sl], -0.5, 1.5, ALU.mult, ALU.add)
                nc.vector.tensor_mul(y_t[sl], y_t[sl], nt[sl])

            def phase_load(i):
                """x prefetch on the sync queue (issued early, async);
                chunk-granular for the pipeline-fill elements."""
                x_t = xp.tile([P, NCH, D], F32, name="x_t")
                xr = x_d[i].rearrange("(c p) d -> p c d", p=P)
                if i == 0:
                    for c in range(NCH):
                        nc.sync.dma_start(x_t[:, c], xr[:, c])
                else:
                    nc.sync.dma_start(x_t[:, 0:2], xr[:, 0:2])
                    nc.sync.dma_start(x_t[:, 2:4], xr[:, 2:4])
                st[i] = dict(x=x_t)

            def phase_A_pre(i):
                """bf16 cast (Pool), RMS stats (ACT square-accum), rsqrt
                (DVE Newton)."""
                x_t = st[i]["x"]
                h8 = h8p.tile([P, NCH, D], BF16, name="h8")
                ms = nstat.tile([P, NCH], F32, name="ms")
                a_t = nstat.tile([P, NCH], F32, name="a_t")
                y_t = nstat.tile([P, NCH], F32, name="y_t")
                nt = nstat.tile([P, NCH], F32, name="nt")
                sqv = nstat.tile([P, D], F32, name="sqv")

                groups = [(0, 2), (2, 4)] if i == 0 else [(0, 4)]
                for lo, hi in groups:
                    if i == 0:
                        for c in range(lo, hi):
                            nc.gpsimd.tensor_copy(h8[:, c], x_t[:, c])
                    else:
                        nc.gpsimd.tensor_copy(
                            h8[:].rearrange("p a b -> p (a b)"),
                            x_t[:].rearrange("p a b -> p (a b)"),
                        )
                    for c in range(lo, hi):
                        nc.scalar.activation(
                            sqv[:], x_t[:, c], ACT.Square,
                            accum_out=ms[:, c : c + 1],
                        )
                    _rsqrt(ms, a_t, y_t, nt, np.s_[:, lo:hi])
                st[i]["h8"] = h8
                st[i]["y"] = y_t

            def phase_A_tp(i, half=None):
                """diag build (Pool), transpose+scale via PE (diag moving
                operand), fp8 evict (DVE), in dc-halves (half=0/1 emits one
                half; None emits both)."""
                h8, y_t = st[i]["h8"], st[i]["y"]
                if half in (None, 0):
                    diagt = dgp.tile([P, NCH, P], BF16, name="diagt")
                    st[i]["diag"] = diagt
                    for c in range(NCH):
                        nc.gpsimd.tensor_scalar_mul(
                            diagt[:, c], ident[:], y_t[:, c : c + 1]
                        )
                    st[i]["hT"] = htp.tile([P, DCH, N], FP8, name="hT")
                diagt, hT = st[i]["diag"], st[i]["hT"]
                halves = (0, 1) if half is None else (half,)
                for dh in halves:
                    tp = bp.tile([P, 2, N], F32, name="tp", tag="b")
                    for c in range(NCH):
                        for j in range(2):
                            dc = 2 * dh + j
                            nc.tensor.matmul(
                                tp[:, j, c * P : (c + 1) * P],
                                lhsT=h8[:, c, dc * P : (dc + 1) * P],
                                rhs=diagt[:, c],
                                start=True,
                                stop=True,
                            )
                    nc.vector.tensor_copy(
                        hT[:, 2 * dh : 2 * dh + 2].rearrange("p a b -> p (a b)"),
                        tp[:].rearrange("p a b -> p (a b)"),
                    )

            def phase_B(i, part=0):
                """proj1 (fp8 DR, 512-wide), silu evictions (ACT, pairs),
                q/k affine (Pool), scores (PE) + evictions (ACT/DVE).
                v-before-u: part=1 emits base+scores+v, part=2 the u pairs
                (part=0 emits everything)."""
                if part in (0, 1):
                    st[i]["uT"] = utp.tile([P, ECH, N], BF16, name="uT")
                    st[i]["v"] = vp.tile([P, NCH, E], FP8, name="v_t")
                    st[i]["qkAB"] = qkabp.tile([P, 4, N], BF16, name="qkAB")
                hT = st[i]["hT"]
                uT, v_t, qkAB = st[i]["uT"], st[i]["v"], st[i]["qkAB"]

                def _proj_ub(ps_slot, mc):
                    for ks in range(2):
                        nc.tensor.matmul(
                            ps_slot,
                            lhsT=w1u_sb[:, ks, :, mc],
                            rhs=hT[:, 2 * ks : 2 * ks + 2, :],
                            start=(ks == 0),
                            stop=(ks == 1),
                            perf_mode=PM.DoubleRow,
                        )

                def _silu(dst, src, mcs):
                    if b1_nonzero:
                        for j, mc in enumerate(mcs):
                            nc.scalar.activation(
                                dst[:, j] if len(mcs) > 1 else dst,
                                src[:, j],
                                SILU_FUNC,
                                bias=b1t_sb[:, mc : mc + 1],
                                scale=1.0 / W1_SCALE,
                            )
                    else:
                        nc.scalar.activation(
                            dst, src[:] if len(mcs) > 1 else src[:, 0],
                            SILU_FUNC, scale=1.0 / W1_SCALE,
                        )

                def _v_pair(nn):
                    ps = bp.tile([P, 2, 512], F32, name="ps", tag="b")
                    for eh in range(2):
                        for ks in range(2):
                            nc.tensor.matmul(
                                ps[:, eh],
                                lhsT=hT[:, 2 * ks : 2 * ks + 2, nn * P : (nn + 1) * P],
                                rhs=w1v_sb[:, ks, :, eh],
                                start=(ks == 0),
                                stop=(ks == 1 and not b1_nonzero),
                                perf_mode=PM.DoubleRow,
                            )
                        if b1_nonzero:
                            nc.tensor.matmul(
                                ps[:, eh],
                                lhsT=ones_sb[:, :],
                                rhs=b1v_sb[:, eh * 512 : (eh + 1) * 512],
                                start=False,
                                stop=True,
                            )
                    nc.scalar.activation(
                        v_t[:, nn],
                        ps[:].rearrange("p a b -> p (a b)"),
                        SILU_FUNC,
                        scale=1.0 / W1_SCALE,
                    )

                def _u_pair(pair):
                    mc = 2 * pair
                    ps = bp.tile([P, 2, 512], F32, name="ps", tag="b")
                    _proj_ub(ps[:, 0], mc)
                    _proj_ub(ps[:, 1], mc + 1)
                    _silu(uT[:, mc : mc + 2], ps, [mc, mc + 1])

                if part in (0, 1):
                    # base first: the score path overlaps the projections
                    baseT = basep.tile([P, N], BF16, name="baseT")
                    qT = basep.tile([P, N], FP8, name="qT")
                    kT = basep.tile([P, N], FP8, name="kT")
                    ps = bp.tile([P, 2, 512], F32, name="ps", tag="b")
                    _proj_ub(ps[:, 0], UBCH - 1)
                    _silu(baseT, ps, [UBCH - 1])
                    nc.gpsimd.tensor_scalar(
                        qT[:], baseT[:], qkg_sb[:, 0:1], qkg_sb[:, 1:2], ALU.mult, ALU.add
                    )
                    nc.gpsimd.tensor_scalar(
                        kT[:], baseT[:], qkg_sb[:, 2:3], qkg_sb[:, 3:4], ALU.mult, ALU.add
                    )
                    _v_pair(0)
                    scs = []
                    for half in range(2):
                        sc = bp.tile([P, 2, N], F32, name="sc", tag="b")
                        scs.append(sc)
                        for mc2 in range(2):
                            nc.tensor.matmul(
                                sc[:, mc2],
                                lhsT=kT[:, (2 * half + mc2) * P : (2 * half + mc2 + 1) * P],
                                rhs=qT[:],
                                start=True,
                                stop=True,
                            )
                    st[i]["scs"] = scs
                    # scaled score eviction A on ACT (B goes to DVE later)
                    nc.scalar.activation(
                        qkAB[:, 0:2], scs[0][:], ACT.Copy, scale=SQ_SCALE
                    )
                if part in (0, 2):
                    for nn in (1, 2, 3):
                        _v_pair(nn)
                if part in (0, 3):
                    for pair in range(4):
                        _u_pair(pair)

            def phase_qkB(i):
                """score eviction half B on DVE (emitted after C_av so the
                DVE queue head never blocks on the score matmuls)."""
                nc.vector.tensor_scalar_mul(
                    st[i]["qkAB"][:, 2:4].rearrange("p a b -> p (a b)"),
                    st[i]["scs"][1][:].rearrange("p a b -> p (a b)"),
                    SQ_SCALE,
                )

            def phase_R(i, last=False):
                """rope muls (Pool/DVE), z/relu (DVE), squared-relu mult
                (Pool; split across DVE+Pool for the last element so its
                tail chain is as short as possible)."""
                qkAB = st[i]["qkAB"]
                t_a = ropep.tile([P, 4, N], BF16, name="t_a", tag="ta")
                t_b = ropep.tile([P, 4, N], BF16, name="t_b", tag="tb")
                z = ropep.tile([P, 4, N], BF16, name="z", tag="z")
                zr = ropep.tile([P, 4, N], BF16, name="zr", tag="zr")
                kernelT = ktp.tile([P, 4, N], FP8, name="kernelT")
                # cs1 = [c0,c1,s0,s1], cs2 = [s0,s1,c0,c1]
                if last:
                    # halves split across engines: the A-dependent parts can
                    # run before the (later) B eviction lands
                    nc.vector.tensor_mul(t_a[:, 0:2], qkAB[:, 0:2], cs1_sb[:, 0:2])
                    nc.gpsimd.tensor_mul(t_b[:, 0:2], qkAB[:, 0:2], cs2_sb[:, 0:2])
                    nc.vector.tensor_mul(t_a[:, 2:4], qkAB[:, 2:4], cs1_sb[:, 2:4])
                    nc.gpsimd.tensor_mul(t_b[:, 2:4], qkAB[:, 2:4], cs2_sb[:, 2:4])
                else:
                    nc.gpsimd.tensor_mul(t_a[:], qkAB[:], cs1_sb[:])
                    nc.gpsimd.tensor_mul(t_b[:], qkAB[:], cs2_sb[:])
                # z_lo = A*cos - B*sin ; z_hi = B*cos + A*sin
                # kernelT = relu(z)*z = relu(z)^2  (z carries 2^18 -> 2^36)
                nc.vector.tensor_sub(z[:, 0:2], t_a[:, 0:2], t_a[:, 2:4])
                nc.vector.tensor_scalar_max(zr[:, 0:2], z[:, 0:2], 0.0)
                nc.gpsimd.tensor_mul(kernelT[:, 0:2], z[:, 0:2], zr[:, 0:2])
                nc.vector.tensor_add(z[:, 2:4], t_b[:, 2:4], t_b[:, 0:2])
                nc.vector.tensor_scalar_max(zr[:, 2:4], z[:, 2:4], 0.0)
                (nc.vector if last else nc.gpsimd).tensor_mul(
                    kernelT[:, 2:4], z[:, 2:4], zr[:, 2:4]
                )
                st[i]["kernelT"] = kernelT

            def phase_C_av(i, qkb_of=None):
                """AV (fp8 DR) + gating (DVE TT) in e-pairs -> gT fp8.
                qkb_of: element whose score-B eviction is interleaved after
                the second gating (keeps DVE busy, unblocks rope early)."""
                uT, v_t, kernelT = st[i]["uT"], st[i]["v"], st[i]["kernelT"]
                gT = gtp.tile([P, ECH, N], FP8, name="gT")
                for pair in range(4):
                    if pair == 2 and qkb_of is not None:
                        phase_qkB(qkb_of)
                    av = cp.tile([P, 2, N], F32, name="av", tag="c")
                    for sub in range(2):
                        ec = 2 * pair + sub
                        for ks in range(2):
                            nc.tensor.matmul(
                                av[:, sub],
                                lhsT=v_t[:, 2 * ks : 2 * ks + 2, ec * P : (ec + 1) * P],
                                rhs=kernelT[:, 2 * ks : 2 * ks + 2, :],
                                start=(ks == 0),
                                stop=(ks == 1),
                                perf_mode=PM.DoubleRow,
                            )
                    nc.vector.tensor_mul(
                        gT[:, 2 * pair : 2 * pair + 2],
                        av[:],
                        uT[:, 2 * pair : 2 * pair + 2],
                    )
                st[i]["gT"] = gT

            def phase_C_out(i, last=False, act_evict=False):
                """out2 (fp8 DR) + descale (DVE TS) + shortcut add (Pool),
                store; nn-pair halves (quarters for the last element)."""
                gT, x_t = st[i]["gT"], st[i]["x"]
                o_t = op.tile([P, NCH, D], F32, name="o_t")
                orr = out_d[i].rearrange("(c p) d -> p c d", p=P)
                for half in range(2):
                    ps = cp.tile([P, 2, 512], F32, name="ps", tag="c")
                    for j in range(2):
                        nn = 2 * half + j
                        for ks in range(4):
                            nc.tensor.matmul(
                                ps[:, j],
                                lhsT=gT[:, 2 * ks : 2 * ks + 2, nn * P : (nn + 1) * P],
                                rhs=w2sb[:, ks],
                                start=(ks == 0),
                                stop=(ks == 3),
                                perf_mode=PM.DoubleRow,
                            )
                        if last:
                            nn_ = np.s_[:, nn : nn + 1]
                            if nn % 2 == 0:
                                nc.scalar.activation(
                                    o_t[nn_].rearrange("p a b -> p (a b)"),
                                    ps[:, j : j + 1].rearrange("p a b -> p (a b)"),
                                    ACT.Copy,
                                    scale=OUT_DESCALE,
                                )
                            else:
                                nc.vector.tensor_scalar_mul(
                                    o_t[nn_].rearrange("p a b -> p (a b)"),
                                    ps[:, j : j + 1].rearrange("p a b -> p (a b)"),
                                    OUT_DESCALE,
                                )
                            nc.gpsimd.tensor_add(
                                o_t[nn_].rearrange("p a b -> p (a b)"),
                                o_t[nn_].rearrange("p a b -> p (a b)"),
                                x_t[nn_].rearrange("p a b -> p (a b)"),
                            )
                            (nc.sync if nn % 2 == 0 else nc.scalar).dma_start(
                                orr[nn_], o_t[nn_]
                            )
                    if not last:
                        sl = np.s_[:, 2 * half : 2 * half + 2]
                        if act_evict:
                            nc.scalar.activation(
                                o_t[sl].rearrange("p a b -> p (a b)"),
                                ps[:].rearrange("p a b -> p (a b)"),
                                ACT.Copy,
                                scale=OUT_DESCALE,
                            )
                        else:
                            nc.vector.tensor_scalar_mul(
                                o_t[sl].rearrange("p a b -> p (a b)"),
                                ps[:].rearrange("p a b -> p (a b)"),
                                OUT_DESCALE,
                            )
                        nc.gpsimd.tensor_add(
                            o_t[sl].rearrange("p a b -> p (a b)"),
                            o_t[sl].rearrange("p a b -> p (a b)"),
                            x_t[sl].rearrange("p a b -> p (a b)"),
                        )
                        nc.sync.dma_start(orr[sl], o_t[sl])
                del st[i]

            # software pipeline, lag-2 with a compressed tail: the last
            # two elements' C phases are offloaded (ACT evictions) and
            # interleaved into B(3).
            phase_load(0)
            load_consts(1)
            phase_load(1)
            load_consts(2)
            phase_A_pre(0)
            phase_A_tp(0)
            phase_load(2)
            load_consts(3)
            for i in range(PER):
                last = i == PER - 1
                if i >= 2:
                    phase_C_av(i - 2)
                phase_B(i, part=1)          # base, affine, v1, scores, qkA
                phase_qkB(i)
                phase_B(i, part=2)          # v2..v4
                if i == 1:
                    phase_R(0)
                elif i >= 2:
                    phase_R(i - 1)
                if i >= 2:
                    phase_C_out(i - 2)
                if last:
                    phase_C_av(2)
                phase_B(i, part=3)          # u pairs
                if i + 1 < PER:
                    phase_A_pre(i + 1)
                    phase_A_tp(i + 1)
                if i == 0:
                    phase_load(3)
            phase_R(PER - 1, last=True)
            phase_C_out(PER - 2, act_evict=True)
            phase_C_av(PER - 1)
            phase_C_out(PER - 1, last=True)

    return nc


def _legalize_sync_waits(nc: bass.Bass) -> bass.Bass:
    """Split excess semaphore waits onto standalone EventSemaphore
    instructions: walrus's per-instruction sync-command slots fit only one
    wait (+update) for DVE/ACT/Pool structs and two for Matmult."""
    import bass_rust

    for f in nc.m.functions:
        for blk in f.blocks:
            insts = blk.instructions
            out = []
            changed = False
            for inst in insts:
                si = getattr(inst, "sync_info", None)
                waits = list(si.on_wait) if si is not None else []
                kind = type(inst).__name__
                if kind == "InstEventSemaphore" or not waits:
                    out.append(inst)
                    continue
                keep = 1
                if len(waits) > keep:
                    extra = waits[keep:]
                    for j in range(0, len(extra), 2):
                        ev = mybir.InstEventSemaphore(
                            name=f"W{j}-{inst.name}", ins=[], outs=[]
                        )
                        ev.engine = inst.engine
                        ev.sync_info = bass_rust.SyncInfo(
                            on_wait=extra[j : j + 2], on_update=[]
                        )
                        out.append(ev)
                    inst.sync_info = bass_rust.SyncInfo(
                        on_wait=waits[:keep], on_update=list(si.on_update)
                    )
                    changed = True
                out.append(inst)
            if changed:
                blk.instructions = out
    return nc


_PROGRAM_CACHE: dict = {}


def _get_program(b1_nonzero: bool) -> bass.Bass:
    key = b1_nonzero
    if key not in _PROGRAM_CACHE:
        _PROGRAM_CACHE[key] = _build_program(b1_nonzero)
    return _PROGRAM_CACHE[key]


def _prep_inputs(inputs):
    fp8 = mybir.dt.np(FP8)
    bf16 = mybir.dt.np(BF16)
    x = np.ascontiguousarray(np.asarray(inputs["x"], np.float32))
    W1 = np.asarray(inputs["W1"], np.float32)
    b1 = np.asarray(inputs["b1"], np.float32)
    W2 = np.asarray(inputs["W2"], np.float32)
    b2 = np.asarray(inputs["b2"], np.float32)
    gamma = np.asarray(inputs["gamma"], np.float32)
    beta = np.asarray(inputs["beta"], np.float32)
    norm_scale = float(np.asarray(inputs["norm_scale"]))

    B = x.shape[0]
    assert x.shape == (B, N, D) and B == CORES * PER, x.shape

    w1s = W1 * (norm_scale * W1_SCALE)  # [512, 2176], fp8-ranged
    # u + base columns (1024 + 128), DoubleRow stationary layout
    # k = (2*ks + t)*128 + p ; m = mc*128 + j
    w1ub = np.concatenate([w1s[:, :E], w1s[:, 2 * E :]], axis=1)  # [512, 1152]
    w1u = np.ascontiguousarray(
        w1ub.reshape(2, 2, P, UBCH, P).transpose(2, 0, 1, 3, 4).astype(fp8).view(np.uint8)
    )
    # v columns, DoubleRow moving layout: [p, ks, t, eh, 512]
    w1v = np.ascontiguousarray(
        w1s[:, E : 2 * E].reshape(2, 2, P, 2, 512).transpose(2, 0, 1, 3, 4).astype(fp8).view(np.uint8)
    )
    # W2 [1024, 512] DoubleRow moving: k = (2*ks + t)*128 + p
    w2r = np.ascontiguousarray(
        (W2 * W2_SCALE).reshape(4, 2, P, 512).transpose(2, 0, 1, 3).astype(fp8).view(np.uint8)
    )
    b1t = np.ascontiguousarray(b1.reshape(PROJ // P, P).T, np.float32)
    b1tu = np.ascontiguousarray(
        np.concatenate([b1t[:, : E // P], b1t[:, 2 * E // P :]], axis=1), np.float32
    )
    # q gets 1/MAX_LEN folded in via the 2^23 descale chain; the affine
    # coefficients are scaled by QK_LAM for fp8 range
    qkg = np.ascontiguousarray(
        np.stack(
            [gamma[0] * QK_LAM, beta[0] * QK_LAM, gamma[1] * QK_LAM, beta[1] * QK_LAM],
            axis=1,
        ),
        np.float32,
    )

    pos = np.arange(N, dtype=np.float32)
    half = N // 2
    inv_freq = (10000.0 ** (-np.arange(half, dtype=np.float32) / np.float32(half))).astype(np.float32)
    sinusoid = (pos[:, None] * inv_freq[None, :]).astype(np.float32)  # [n, half]
    cosT = np.cos(sinusoid).astype(np.float32).T.reshape(2, P, N)  # [2, p, n]
    sinT = np.sin(sinusoid).astype(np.float32).T.reshape(2, P, N)
    cs1 = np.ascontiguousarray(
        np.concatenate([cosT, sinT], 0).transpose(1, 0, 2).astype(bf16)
    ).view(np.uint16)
    cs2 = np.ascontiguousarray(
        np.concatenate([sinT, cosT], 0).transpose(1, 0, 2).astype(bf16)
    ).view(np.uint16)

    b1_nonzero = bool(np.any(b1))
    xin = x.reshape(CORES, PER, N, D)

    in_maps = []
    for c in range(CORES):
        m = dict(
            x=np.ascontiguousarray(xin[c]),
            w1u=w1u, w1v=w1v, w2=w2r, b1t=b1tu, qkg=qkg, cs1=cs1, cs2=cs2,
        )
        if b1_nonzero:
            m["b1v"] = np.ascontiguousarray(
                (b1[E : 2 * E] * W1_SCALE).reshape(1, E).astype(fp8).view(np.uint8)
            )
        in_maps.append(m)
    return in_maps, b1_nonzero, b2


def _ensure_axon_hook_stub():
    try:
        import antenv.axon_hooks  # noqa: F401
    except ImportError:
        import types
        import antenv
        stub = types.ModuleType("antenv.axon_hooks")
        stub.get_axon_ntff_profile_hook = lambda: None
        sys.modules["antenv.axon_hooks"] = stub
        antenv.axon_hooks = stub


def _run(inputs, trace=False):
    _ensure_axon_hook_stub()
    in_maps, b1nz, b2 = _prep_inputs(inputs)
    nc = _get_program(b1nz)
    if not getattr(nc, "_sync_legalized", False):
        _legalize_sync_waits(nc)
        nc._sync_legalized = True
    res = run_bass_kernel_spmd(nc, in_maps, core_ids=list(range(CORES)), trace=trace)
    out = np.concatenate([r["out"] for r in res.results], axis=0).reshape(CORES * PER, N, D)
    out = out.astype(np.float32)
    if np.any(b2):
        out = out + b2  # zero in the graded setup; kept for generality
    return out, res


def kernel(**inputs) -> np.ndarray:
    out, _ = _run(inputs)
    return out
